# revision 9
# baseline (speedup 1.0000x reference)
"""CNN char encoder (conv widths 1/2/3 -> tanh -> max over time -> highway)
as a Bass/Tile kernel for 8 Trainium2 NeuronCores.

Data-parallel over the 4096 = 32*128 flattened words; 512 words/core; all
weights replicated. Feature-major on chip ([feature_partition, word]); host
transposes back.

Input ships in the fp16 "pair" layout: column u holds char positions
(2u -> rows 0..49, 2u+1 -> rows 64..113); all conv matmuls contract the
full 128 partitions (K<128 streams at ~half rate on this hw; mixed
base-partition accumulation groups crash the device), with zero weight
rows where a tap doesn't apply.  Position t is one PSUM "slot"; a 4-bank
PSUM quad holds 4 consecutive positions (2 pair-columns), so conv epilogue
ops run at FD=2048 (half the per-op overhead of the 2-bank scheme).

Epilogue (max over time), per quad, statically routed:
  ACT route: tanh(+bias) PSUM -> fp16 slots; running slot-max on DVE or
             GpSimd in fp16 (2x mode; GPS only contends with 2-port DVE
             ops, and DVE's conv work is 1-port PSUM reads);
  DVE route: fp16 running slot-max straight from PSUM (1x), single tanh
             at the end.
Routes are checkerboarded across banks/groups so ACT, DVE and GPS all stay
busy in every group; 2 fold ops + a cross-route merge per bank finish
feat.  ~10 zero dummy matmuls at t~1us keep the PE HAM clock warm through
the initial input-DMA window; x arrives in per-pair-column chunks so conv
starts ~5us in and never starves.

Highway: fp16 matmuls, 2 out-tiles per 4-bank PSUM quad; epilogue
t*(h-f)+f in fp16 processed as ot-pairs (FD=1024); fp16 output DMA (host
upcasts to fp32).
"""

import numpy as np

import concourse.bass as bass
import concourse.tile as tile
from concourse import bacc, mybir
from concourse.bass_utils import run_bass_kernel_spmd

F32 = mybir.dt.float32
F16 = mybir.dt.float16
ACTF = mybir.ActivationFunctionType

N_CORES = 8
B, S, L, C = 32, 128, 20, 50
NW = B * S               # 4096 words total
WPC = NW // N_CORES      # 512 words per core
U = L // 2               # 10 pair-columns per word
D1 = 64                  # partition base of the odd-position block
OUT_DIM = 768


def build_nc(n_dev=N_CORES):
    nc = bacc.Bacc(
        "TRN2", target_bir_lowering=False, debug=False, num_devices=n_dev
    )

    xp = nc.dram_tensor("xp", [128, U * WPC], F16, kind="ExternalInput")
    w1e = nc.dram_tensor("w1e", [128, 128], F16, kind="ExternalInput")
    w1o = nc.dram_tensor("w1o", [128, 128], F16, kind="ExternalInput")
    w2p = nc.dram_tensor("w2p", [128, 128], F16, kind="ExternalInput")
    w2z = nc.dram_tensor("w2z", [128, 128], F16, kind="ExternalInput")
    w2o = nc.dram_tensor("w2o", [128, 128], F16, kind="ExternalInput")
    w3a = nc.dram_tensor("w3a", [128, 512], F16, kind="ExternalInput")
    w3t2 = nc.dram_tensor("w3t2", [128, 512], F16, kind="ExternalInput")
    w3o = nc.dram_tensor("w3o", [128, 512], F16, kind="ExternalInput")
    w3d = nc.dram_tensor("w3d", [128, 512], F16, kind="ExternalInput")
    biasp = nc.dram_tensor("biasp", [128, 18], F32, kind="ExternalInput")
    whp = nc.dram_tensor("whp", [128, 36 * 128], F16, kind="ExternalInput")
    wtp = nc.dram_tensor("wtp", [128, 36 * 128], F16, kind="ExternalInput")
    out = nc.dram_tensor("out", [OUT_DIM, WPC], F16, kind="ExternalOutput")

    with tile.TileContext(nc) as tc:
        with (
            tc.tile_pool(name="singles", bufs=1) as singles,
            tc.tile_pool(name="psum", bufs=2, space="PSUM") as psum,
            tc.tile_pool(name="scrp", bufs=3) as scrp,
            tc.tile_pool(name="gscr", bufs=2) as gscr,
            tc.tile_pool(name="hwt", bufs=2) as hwt,
        ):
            # --- PE warmup scratch; memset leads the gpsimd queue
            wscr = singles.tile([128, WPC], F16)
            nc.gpsimd.memset(wscr, 0.0)

            # --- DMAs.  The first conv quads need w3a/w3t2 + x cols 0..2,
            # so those weights lead the two fast HWDGE queues; the rest of
            # the conv weights + bias go on gpsimd(SWDGE); x row-halves
            # stream per pair column; big highway weights last.
            sw3a = singles.tile([128, 512], F16)
            nc.sync.dma_start(out=sw3a, in_=w3a.ap())
            sw3t2 = singles.tile([128, 512], F16)
            nc.scalar.dma_start(out=sw3t2, in_=w3t2.ap())
            sbias = singles.tile([128, 18], F32)
            nc.gpsimd.dma_start(out=sbias, in_=biasp.ap())
            sw3o = singles.tile([128, 512], F16)
            nc.gpsimd.dma_start(out=sw3o, in_=w3o.ap())
            sw3d = singles.tile([128, 512], F16)
            nc.gpsimd.dma_start(out=sw3d, in_=w3d.ap())
            sw1e = singles.tile([128, 128], F16)
            nc.gpsimd.dma_start(out=sw1e, in_=w1e.ap())
            sw1o = singles.tile([128, 128], F16)
            nc.gpsimd.dma_start(out=sw1o, in_=w1o.ap())
            sw2p = singles.tile([128, 128], F16)
            nc.gpsimd.dma_start(out=sw2p, in_=w2p.ap())
            sw2z = singles.tile([128, 128], F16)
            nc.gpsimd.dma_start(out=sw2z, in_=w2z.ap())
            sw2o = singles.tile([128, 128], F16)
            nc.gpsimd.dma_start(out=sw2o, in_=w2o.ap())

            sx = singles.tile([128, U, WPC], F16)
            for u in range(U):
                nc.sync.dma_start(
                    out=sx[0:64, u, :],
                    in_=xp.ap()[0:64, u * WPC : (u + 1) * WPC],
                )
                nc.scalar.dma_start(
                    out=sx[64:128, u, :],
                    in_=xp.ap()[64:128, u * WPC : (u + 1) * WPC],
                )
            swh = singles.tile([128, 36 * 128], F16)
            nc.sync.dma_start(out=swh, in_=whp.ap())
            swt = singles.tile([128, 36 * 128], F16)
            nc.scalar.dma_start(out=swt, in_=wtp.ap())

            # --- PE warmup: dummy matmuls so HAM un-throttles during the
            # input-DMA window and conv starts at full clock
            wq = psum.tile([128, 2, WPC], F32, name="cq", bufs=4)
            for i in range(10):
                nc.tensor.matmul(
                    wq[:, i % 2, :], wscr[:, 0:128], wscr,
                    start=True, stop=True,
                )

            featall = singles.tile([128, 6, WPC], F16)

            # accumulators (fp16): ACT-route post-tanh, DVE-route pre-tanh
            acc_a = {}
            for key in ["w1", "w2", 0, 1, 2, 3]:
                acc_a[key] = singles.tile([128, 4, WPC], F16,
                                          name=f"aa_{key}")
            acc_d = {}
            for b in range(4):
                acc_d[b] = singles.tile([128, 4, WPC], F16, name=f"ad_{b}")

            first_a = {k: True for k in acc_a}
            first_d = {b: True for b in acc_d}

            def bias_col(key):
                col = {"w1": 0, "w2": 1}.get(key, None)
                if col is None:
                    col = 2 + key
                return sbias[:, col : col + 1]

            def col(u):
                return sx[:, u, :]

            def quad_mms(key, g):
                """list per slot of [(lhsT, rhs), ...] accumulation groups
                for positions 4g..4g+3 of this bank."""
                slots = []
                for s in range(4):
                    t = 4 * g + s
                    u, d = t // 2, t % 2
                    if key == "w1":
                        if t >= 20:
                            continue
                        slots.append([(sw1e if d == 0 else sw1o, col(u))])
                    elif key == "w2":
                        if t >= 19:
                            continue
                        if d == 0:
                            slots.append([(sw2p, col(u))])
                        else:
                            slots.append([(sw2z, col(u)), (sw2o, col(u + 1))])
                    else:
                        if t >= 18:
                            continue
                        cs = slice(key * 128, key * 128 + 128)
                        if d == 0:
                            slots.append([(sw3a[:, cs], col(u)),
                                          (sw3t2[:, cs], col(u + 1))])
                        else:
                            slots.append([(sw3o[:, cs], col(u)),
                                          (sw3d[:, cs], col(u + 1))])
                return slots

            def is_act_route(key, g):
                if key in ("w1", "w2"):
                    return True
                # w3: 2 ACT + 2 DVE full quads per bank, checkerboarded;
                # g4 (half quad) goes DVE to keep ACT off the critical path
                return g < 4 and (g + key) % 2 == 1

            def do_quad(key, g):
                """positions 4g..4g+3 as two 2-bank PSUM duos (finer PE
                pipelining); ACT-route taxes still run at quad width."""
                slots = quad_mms(key, g)
                if not slots:
                    return
                P = len(slots)
                act = is_act_route(key, g)
                scr = None
                if act and not first_a[key]:
                    scr = scrp.tile([128, 4, WPC], F16, name="scr", bufs=3)
                for h in (0, 1):
                    duo = slots[2 * h : 2 * h + 2]
                    if not duo:
                        continue
                    Pd = len(duo)
                    cq = psum.tile([128, 2, WPC], F32, name="cq", bufs=4)
                    for s, mms in enumerate(duo):
                        for i, (lh, rh) in enumerate(mms):
                            nc.tensor.matmul(
                                cq[:, s, :], lh, rh,
                                start=(i == 0), stop=(i == len(mms) - 1),
                            )
                    view = cq[:, 0:Pd, :]
                    sl = slice(2 * h, 2 * h + Pd)
                    if act:
                        dest = acc_a[key] if scr is None else scr
                        nc.scalar.activation(
                            dest[:, sl, :], view, ACTF.Tanh,
                            bias=bias_col(key),
                        )
                    else:
                        accd = acc_d[key]
                        if first_d[key]:
                            nc.vector.tensor_copy(accd[:, sl, :], view)
                        else:
                            nc.vector.tensor_max(
                                accd[:, sl, :], accd[:, sl, :], view
                            )
                if act:
                    if scr is not None:
                        nc.vector.tensor_max(
                            acc_a[key][:, 0:P, :],
                            acc_a[key][:, 0:P, :], scr[:, 0:P, :],
                        )
                    first_a[key] = False
                else:
                    first_d[key] = False

            def fold4(acc, dest):
                t = gscr.tile([128, 2, WPC], F16, name="fold2", bufs=2)
                nc.vector.tensor_max(t, acc[:, 0:2, :], acc[:, 2:4, :])
                nc.vector.tensor_max(dest, t[:, 0, :], t[:, 1, :])

            def merge_bank(key):
                if key == "w1":
                    fold4(acc_a["w1"], featall[:, 0, :])
                elif key == "w2":
                    fold4(acc_a["w2"], featall[:, 1, :])
                else:
                    b = key
                    fa = gscr.tile([128, WPC], F16, name="fa", bufs=2)
                    fold4(acc_a[b], fa)
                    fd = gscr.tile([128, WPC], F16, name="fd", bufs=2)
                    fold4(acc_d[b], fd)
                    pd = gscr.tile([128, WPC], F16, name="pd", bufs=2)
                    nc.scalar.activation(pd, fd, ACTF.Tanh,
                                         bias=bias_col(b))
                    nc.vector.tensor_max(featall[:, 2 + b, :], fa, pd)

            BANK_ORDER = [0, 1, "w1", 2, 3, "w2"]
            # last group: w1/w2 first and merge each bank eagerly so the
            # highway's early kt accumulation layers unblock sooner
            for g in range(4):
                for key in BANK_ORDER:
                    do_quad(key, g)
            for key in ["w1", "w2", 0, 1, 2, 3]:
                do_quad(key, 4)
                merge_bank(key)

            # --- highway: 2 waves of 3 out-tiles, kt-MAJOR matmul order so
            # the PE streams early-kt layers while the last conv banks are
            # still merging (feat_kt unblock progressively)
            for w in range(2):
                ots = [3 * w, 3 * w + 1, 3 * w + 2]
                hqs = {ot: psum.tile([128, 2, WPC], F32, name="cq", bufs=4)
                       for ot in ots}
                for kt in range(6):
                    for ot in ots:
                        for sl, wsb in enumerate((swh, swt)):
                            blk = (ot * 6 + kt) * 128
                            nc.tensor.matmul(
                                hqs[ot][:, sl, :], wsb[:, blk : blk + 128],
                                featall[:, kt, :],
                                start=(kt == 0), stop=(kt == 5),
                            )
                for ot in ots:
                    hq = hqs[ot]
                    h1 = hwt.tile([128, WPC], F16, name="h1", bufs=3)
                    t1 = hwt.tile([128, WPC], F16, name="t1", bufs=3)
                    nc.scalar.activation(
                        h1, hq[:, 0, :], ACTF.Relu,
                        bias=sbias[:, 6 + ot : 7 + ot],
                    )
                    nc.scalar.activation(
                        t1, hq[:, 1, :], ACTF.Sigmoid,
                        bias=sbias[:, 12 + ot : 13 + ot],
                    )
                    fcur = featall[:, ot, :]
                    o1 = hwt.tile([128, WPC], F16, name="o1", bufs=3)
                    nc.vector.tensor_sub(h1, h1, fcur)
                    nc.vector.tensor_mul(h1, t1, h1)
                    nc.vector.tensor_add(o1, h1, fcur)
                    # split the 131KB out-DMA across both HWDGE queues
                    nc.sync.dma_start(
                        out=out.ap()[ot * 128 : ot * 128 + 64, :],
                        in_=o1[0:64, :],
                    )
                    nc.scalar.dma_start(
                        out=out.ap()[ot * 128 + 64 : (ot + 1) * 128, :],
                        in_=o1[64:128, :],
                    )

    nc.compile()
    return nc


def pack_inputs(ts10_input, conv_w0, conv_b0, conv_w1, conv_b1, conv_w2,
                conv_b2, wh_w, wh_b, wt_w, wt_b):
    f = np.float32
    bf = np.float16

    def padded(top, bottom, ncols):
        arr = np.zeros((128, ncols), f)
        if top is not None:
            arr[0:C] = top
        if bottom is not None:
            arr[D1 : D1 + C] = bottom
        return arr.astype(bf)

    X = np.ascontiguousarray(ts10_input, dtype=f).reshape(NW, L, C)
    w1t = conv_w0[:, :, 0].T
    shared = dict(
        w1e=padded(w1t, None, 128),
        w1o=padded(None, w1t, 128),
        w2p=padded(conv_w1[:, :, 0].T, conv_w1[:, :, 1].T, 128),
        w2z=padded(None, conv_w1[:, :, 0].T, 128),
        w2o=padded(conv_w1[:, :, 1].T, None, 128),
        w3a=padded(conv_w2[:, :, 0].T, conv_w2[:, :, 1].T, 512),
        w3t2=padded(conv_w2[:, :, 2].T, None, 512),
        w3o=padded(None, conv_w2[:, :, 0].T, 512),
        w3d=padded(conv_w2[:, :, 1].T, conv_w2[:, :, 2].T, 512),
    )
    biasp = np.zeros((128, 18), f)
    biasp[:, 0] = conv_b0
    biasp[:, 1] = conv_b1
    for b in range(4):
        biasp[:, 2 + b] = conv_b2[b * 128 : (b + 1) * 128]
    for ot in range(6):
        biasp[:, 6 + ot] = wh_b[ot * 128 : (ot + 1) * 128]
        biasp[:, 12 + ot] = wt_b[ot * 128 : (ot + 1) * 128]
    shared["biasp"] = biasp
    shared["whp"] = np.ascontiguousarray(
        wh_w.reshape(6, 128, 6, 128).transpose(3, 0, 2, 1).reshape(128, -1)
    ).astype(bf)
    shared["wtp"] = np.ascontiguousarray(
        wt_w.reshape(6, 128, 6, 128).transpose(3, 0, 2, 1).reshape(128, -1)
    ).astype(bf)

    in_maps = []
    for c in range(N_CORES):
        Xc = X[c * WPC : (c + 1) * WPC]                        # [512, 20, 50]
        pair = Xc.reshape(WPC, U, 2, C).transpose(2, 3, 1, 0)  # [2, C, U, 512]
        xpc = np.zeros((128, U * WPC), f)
        xpc[0:C] = pair[0].reshape(C, U * WPC)
        xpc[D1 : D1 + C] = pair[1].reshape(C, U * WPC)
        in_maps.append(dict(xp=xpc.astype(bf), **shared))
    return in_maps


_NC_CACHE = None


def get_nc():
    global _NC_CACHE
    if _NC_CACHE is None:
        _NC_CACHE = build_nc()
    return _NC_CACHE


def kernel(**inputs):
    in_maps = pack_inputs(**{k: np.asarray(v) for k, v in inputs.items()})
    nc = get_nc()
    res = run_bass_kernel_spmd(nc, in_maps, core_ids=list(range(N_CORES)))
    full = np.empty((NW, OUT_DIM), np.float32)
    for c in range(N_CORES):
        full[c * WPC : (c + 1) * WPC] = res.results[c]["out"].T.astype(np.float32)
    return full.reshape(B, S, OUT_DIM)


# revision 13
# speedup vs baseline: 1.0005x; 1.0005x over previous
"""CNN char encoder (conv widths 1/2/3 -> tanh -> max over time -> highway)
as a Bass/Tile kernel for 8 Trainium2 NeuronCores.

Data-parallel over the 4096 = 32*128 flattened words; 512 words/core; all
weights replicated. Feature-major on chip ([feature_partition, word]); host
transposes back.

Input ships in the fp16 "pair" layout: column u holds char positions
(2u -> rows 0..49, 2u+1 -> rows 64..113); all conv matmuls contract the
full 128 partitions (K<128 streams at ~half rate on this hw; mixed
base-partition accumulation groups crash the device), with zero weight
rows where a tap doesn't apply.  Position t is one PSUM "slot"; a 4-bank
PSUM quad holds 4 consecutive positions (2 pair-columns), so conv epilogue
ops run at FD=2048 (half the per-op overhead of the 2-bank scheme).

Epilogue (max over time), per quad, statically routed:
  ACT route: tanh(+bias) PSUM -> fp16 slots; running slot-max on DVE or
             GpSimd in fp16 (2x mode; GPS only contends with 2-port DVE
             ops, and DVE's conv work is 1-port PSUM reads);
  DVE route: fp16 running slot-max straight from PSUM (1x), single tanh
             at the end.
Routes are checkerboarded across banks/groups so ACT, DVE and GPS all stay
busy in every group; 2 fold ops + a cross-route merge per bank finish
feat.  ~10 zero dummy matmuls at t~1us keep the PE HAM clock warm through
the initial input-DMA window; x arrives in per-pair-column chunks so conv
starts ~5us in and never starves.

Highway: fp16 matmuls, 2 out-tiles per 4-bank PSUM quad; epilogue
t*(h-f)+f in fp16 processed as ot-pairs (FD=1024); fp16 output DMA (host
upcasts to fp32).
"""

import numpy as np

import concourse.bass as bass
import concourse.tile as tile
from concourse import bacc, mybir
from concourse.bass_utils import run_bass_kernel_spmd

F32 = mybir.dt.float32
F16 = mybir.dt.float16
ACTF = mybir.ActivationFunctionType

N_CORES = 8
B, S, L, C = 32, 128, 20, 50
NW = B * S               # 4096 words total
WPC = NW // N_CORES      # 512 words per core
U = L // 2               # 10 pair-columns per word
D1 = 64                  # partition base of the odd-position block
OUT_DIM = 768


def build_nc(n_dev=N_CORES):
    nc = bacc.Bacc(
        "TRN2", target_bir_lowering=False, debug=False, num_devices=n_dev
    )

    xp = nc.dram_tensor("xp", [128, U * WPC], F16, kind="ExternalInput")
    w1e = nc.dram_tensor("w1e", [128, 128], F16, kind="ExternalInput")
    w1o = nc.dram_tensor("w1o", [128, 128], F16, kind="ExternalInput")
    w2p = nc.dram_tensor("w2p", [128, 128], F16, kind="ExternalInput")
    w2z = nc.dram_tensor("w2z", [128, 128], F16, kind="ExternalInput")
    w2o = nc.dram_tensor("w2o", [128, 128], F16, kind="ExternalInput")
    w3a = nc.dram_tensor("w3a", [128, 512], F16, kind="ExternalInput")
    w3t2 = nc.dram_tensor("w3t2", [128, 512], F16, kind="ExternalInput")
    w3o = nc.dram_tensor("w3o", [128, 512], F16, kind="ExternalInput")
    w3d = nc.dram_tensor("w3d", [128, 512], F16, kind="ExternalInput")
    biasp = nc.dram_tensor("biasp", [128, 18], F32, kind="ExternalInput")
    whp = nc.dram_tensor("whp", [128, 36 * 128], F16, kind="ExternalInput")
    wtp = nc.dram_tensor("wtp", [128, 36 * 128], F16, kind="ExternalInput")
    out = nc.dram_tensor("out", [OUT_DIM, WPC], F16, kind="ExternalOutput")

    with tile.TileContext(nc) as tc:
        with (
            tc.tile_pool(name="singles", bufs=1) as singles,
            tc.tile_pool(name="psum", bufs=2, space="PSUM") as psum,
            tc.tile_pool(name="scrp", bufs=3) as scrp,
            tc.tile_pool(name="gscr", bufs=2) as gscr,
            tc.tile_pool(name="hwt", bufs=2) as hwt,
        ):
            # --- PE warmup scratch; memset leads the gpsimd queue
            wscr = singles.tile([128, WPC], F16)
            nc.gpsimd.memset(wscr, 0.0)

            # --- DMAs.  The first conv quads need w3a/w3t2 + x cols 0..2,
            # so those weights lead the two fast HWDGE queues; the rest of
            # the conv weights + bias go on gpsimd(SWDGE); x row-halves
            # stream per pair column; big highway weights last.
            sw3a = singles.tile([128, 512], F16)
            sw3t2 = singles.tile([128, 512], F16)
            sbias = singles.tile([128, 18], F32)
            nc.gpsimd.dma_start(out=sbias, in_=biasp.ap())
            sw3o = singles.tile([128, 512], F16)
            nc.gpsimd.dma_start(out=sw3o, in_=w3o.ap())
            sw3d = singles.tile([128, 512], F16)
            nc.gpsimd.dma_start(out=sw3d, in_=w3d.ap())
            sw1e = singles.tile([128, 128], F16)
            nc.gpsimd.dma_start(out=sw1e, in_=w1e.ap())
            sw1o = singles.tile([128, 128], F16)
            nc.gpsimd.dma_start(out=sw1o, in_=w1o.ap())
            sw2p = singles.tile([128, 128], F16)
            nc.gpsimd.dma_start(out=sw2p, in_=w2p.ap())
            sw2z = singles.tile([128, 128], F16)
            nc.gpsimd.dma_start(out=sw2z, in_=w2z.ap())
            sw2o = singles.tile([128, 128], F16)
            nc.gpsimd.dma_start(out=sw2o, in_=w2o.ap())

            # x cols 0..2 first (w1/w2 quads of group 0 need only those),
            # then the w3 full-tap weights split across both queues, then
            # the remaining cols
            sx = singles.tile([128, U, WPC], F16)

            def xcol_dma(u):
                nc.sync.dma_start(
                    out=sx[0:64, u, :],
                    in_=xp.ap()[0:64, u * WPC : (u + 1) * WPC],
                )
                nc.scalar.dma_start(
                    out=sx[64:128, u, :],
                    in_=xp.ap()[64:128, u * WPC : (u + 1) * WPC],
                )

            for u in range(3):
                xcol_dma(u)
            nc.sync.dma_start(out=sw3a[0:64, :], in_=w3a.ap()[0:64, :])
            nc.scalar.dma_start(out=sw3a[64:128, :], in_=w3a.ap()[64:128, :])
            nc.sync.dma_start(out=sw3t2[0:64, :], in_=w3t2.ap()[0:64, :])
            nc.scalar.dma_start(out=sw3t2[64:128, :], in_=w3t2.ap()[64:128, :])
            for u in range(3, U):
                xcol_dma(u)
            swh = singles.tile([128, 36 * 128], F16)
            nc.sync.dma_start(out=swh, in_=whp.ap())
            swt = singles.tile([128, 36 * 128], F16)
            nc.scalar.dma_start(out=swt, in_=wtp.ap())

            # --- PE warmup: dummy matmuls so HAM un-throttles during the
            # input-DMA window and conv starts at full clock
            wq = psum.tile([128, 2, WPC], F32, name="cq", bufs=4)
            for i in range(24):
                nc.tensor.matmul(
                    wq[:, i % 2, :], wscr[:, 0:128], wscr,
                    start=True, stop=True,
                )

            featall = singles.tile([128, 6, WPC], F16)

            # accumulators (fp16): ACT-route post-tanh, DVE-route pre-tanh
            acc_a = {}
            for key in ["w1", "w2", 0, 1, 2, 3]:
                acc_a[key] = singles.tile([128, 4, WPC], F16,
                                          name=f"aa_{key}")
            acc_d = {}
            for b in range(4):
                acc_d[b] = singles.tile([128, 4, WPC], F16, name=f"ad_{b}")

            first_a = {k: True for k in acc_a}
            first_d = {b: True for b in acc_d}

            def bias_col(key):
                col = {"w1": 0, "w2": 1}.get(key, None)
                if col is None:
                    col = 2 + key
                return sbias[:, col : col + 1]

            def col(u):
                return sx[:, u, :]

            def quad_mms(key, g):
                """list per slot of [(lhsT, rhs), ...] accumulation groups
                for positions 4g..4g+3 of this bank."""
                slots = []
                for s in range(4):
                    t = 4 * g + s
                    u, d = t // 2, t % 2
                    if key == "w1":
                        if t >= 20:
                            continue
                        slots.append([(sw1e if d == 0 else sw1o, col(u))])
                    elif key == "w2":
                        if t >= 19:
                            continue
                        if d == 0:
                            slots.append([(sw2p, col(u))])
                        else:
                            slots.append([(sw2z, col(u)), (sw2o, col(u + 1))])
                    else:
                        if t >= 18:
                            continue
                        cs = slice(key * 128, key * 128 + 128)
                        if d == 0:
                            slots.append([(sw3a[:, cs], col(u)),
                                          (sw3t2[:, cs], col(u + 1))])
                        else:
                            slots.append([(sw3o[:, cs], col(u)),
                                          (sw3d[:, cs], col(u + 1))])
                return slots

            def is_act_route(key, g):
                if key in ("w1", "w2"):
                    return True
                # w3: 2 ACT + 2 DVE full quads per bank, checkerboarded;
                # g4 (half quad) goes DVE to keep ACT off the critical path
                return g < 4 and (g + key) % 2 == 1

            def do_quad(key, g):
                """positions 4g..4g+3 as two 2-bank PSUM duos (finer PE
                pipelining); ACT-route taxes still run at quad width."""
                slots = quad_mms(key, g)
                if not slots:
                    return
                P = len(slots)
                act = is_act_route(key, g)
                scr = None
                if act and not first_a[key]:
                    scr = scrp.tile([128, 4, WPC], F16, name="scr", bufs=3)
                for h in (0, 1):
                    duo = slots[2 * h : 2 * h + 2]
                    if not duo:
                        continue
                    Pd = len(duo)
                    cq = psum.tile([128, 2, WPC], F32, name="cq", bufs=4)
                    for s, mms in enumerate(duo):
                        for i, (lh, rh) in enumerate(mms):
                            nc.tensor.matmul(
                                cq[:, s, :], lh, rh,
                                start=(i == 0), stop=(i == len(mms) - 1),
                            )
                    view = cq[:, 0:Pd, :]
                    sl = slice(2 * h, 2 * h + Pd)
                    if act:
                        dest = acc_a[key] if scr is None else scr
                        nc.scalar.activation(
                            dest[:, sl, :], view, ACTF.Tanh,
                            bias=bias_col(key),
                        )
                    else:
                        accd = acc_d[key]
                        if first_d[key]:
                            nc.vector.tensor_copy(accd[:, sl, :], view)
                        else:
                            nc.vector.tensor_max(
                                accd[:, sl, :], accd[:, sl, :], view
                            )
                if act:
                    if scr is not None:
                        nc.vector.tensor_max(
                            acc_a[key][:, 0:P, :],
                            acc_a[key][:, 0:P, :], scr[:, 0:P, :],
                        )
                    first_a[key] = False
                else:
                    first_d[key] = False

            def fold4(acc, dest):
                t = gscr.tile([128, 2, WPC], F16, name="fold2", bufs=2)
                nc.vector.tensor_max(t, acc[:, 0:2, :], acc[:, 2:4, :])
                nc.vector.tensor_max(dest, t[:, 0, :], t[:, 1, :])

            def merge_bank(key):
                if key == "w1":
                    fold4(acc_a["w1"], featall[:, 0, :])
                elif key == "w2":
                    fold4(acc_a["w2"], featall[:, 1, :])
                else:
                    b = key
                    fa = gscr.tile([128, WPC], F16, name="fa", bufs=2)
                    fold4(acc_a[b], fa)
                    fd = gscr.tile([128, WPC], F16, name="fd", bufs=2)
                    fold4(acc_d[b], fd)
                    pd = gscr.tile([128, WPC], F16, name="pd", bufs=2)
                    nc.scalar.activation(pd, fd, ACTF.Tanh,
                                         bias=bias_col(b))
                    nc.vector.tensor_max(featall[:, 2 + b, :], fa, pd)

            BANK_ORDER = [0, 1, "w1", 2, 3, "w2"]
            # group 0 starts with w1/w2 (they only need x cols 0..2 and the
            # small gpsimd-queued weights -> earliest possible real work);
            # last group: w1/w2 first with eager merges so the highway's
            # early kt accumulation layers unblock sooner
            for key in ["w1", "w2", 0, 1, 2, 3]:
                do_quad(key, 0)
            for g in range(1, 4):
                for key in BANK_ORDER:
                    do_quad(key, g)
            for key in ["w1", "w2", 0, 1, 2, 3]:
                do_quad(key, 4)
                merge_bank(key)

            # --- highway: 2 waves of 3 out-tiles, kt-MAJOR matmul order so
            # the PE streams early-kt layers while the last conv banks are
            # still merging (feat_kt unblock progressively)
            for w in range(2):
                ots = [3 * w, 3 * w + 1, 3 * w + 2]
                hqs = {ot: psum.tile([128, 2, WPC], F32, name="cq", bufs=4)
                       for ot in ots}
                for kt in range(6):
                    for ot in ots:
                        for sl, wsb in enumerate((swh, swt)):
                            blk = (ot * 6 + kt) * 128
                            nc.tensor.matmul(
                                hqs[ot][:, sl, :], wsb[:, blk : blk + 128],
                                featall[:, kt, :],
                                start=(kt == 0), stop=(kt == 5),
                            )
                for ot in ots:
                    hq = hqs[ot]
                    h1 = hwt.tile([128, WPC], F16, name="h1", bufs=3)
                    t1 = hwt.tile([128, WPC], F16, name="t1", bufs=3)
                    nc.scalar.activation(
                        h1, hq[:, 0, :], ACTF.Relu,
                        bias=sbias[:, 6 + ot : 7 + ot],
                    )
                    nc.scalar.activation(
                        t1, hq[:, 1, :], ACTF.Sigmoid,
                        bias=sbias[:, 12 + ot : 13 + ot],
                    )
                    fcur = featall[:, ot, :]
                    o1 = hwt.tile([128, WPC], F16, name="o1", bufs=3)
                    nc.vector.tensor_sub(h1, h1, fcur)
                    nc.vector.tensor_mul(h1, t1, h1)
                    nc.vector.tensor_add(o1, h1, fcur)
                    # split the 131KB out-DMA across both HWDGE queues
                    nc.sync.dma_start(
                        out=out.ap()[ot * 128 : ot * 128 + 64, :],
                        in_=o1[0:64, :],
                    )
                    nc.scalar.dma_start(
                        out=out.ap()[ot * 128 + 64 : (ot + 1) * 128, :],
                        in_=o1[64:128, :],
                    )

    nc.compile()
    return nc


def pack_inputs(ts10_input, conv_w0, conv_b0, conv_w1, conv_b1, conv_w2,
                conv_b2, wh_w, wh_b, wt_w, wt_b):
    f = np.float32
    bf = np.float16

    def padded(top, bottom, ncols):
        arr = np.zeros((128, ncols), f)
        if top is not None:
            arr[0:C] = top
        if bottom is not None:
            arr[D1 : D1 + C] = bottom
        return arr.astype(bf)

    X = np.ascontiguousarray(ts10_input, dtype=f).reshape(NW, L, C)
    w1t = conv_w0[:, :, 0].T
    shared = dict(
        w1e=padded(w1t, None, 128),
        w1o=padded(None, w1t, 128),
        w2p=padded(conv_w1[:, :, 0].T, conv_w1[:, :, 1].T, 128),
        w2z=padded(None, conv_w1[:, :, 0].T, 128),
        w2o=padded(conv_w1[:, :, 1].T, None, 128),
        w3a=padded(conv_w2[:, :, 0].T, conv_w2[:, :, 1].T, 512),
        w3t2=padded(conv_w2[:, :, 2].T, None, 512),
        w3o=padded(None, conv_w2[:, :, 0].T, 512),
        w3d=padded(conv_w2[:, :, 1].T, conv_w2[:, :, 2].T, 512),
    )
    biasp = np.zeros((128, 18), f)
    biasp[:, 0] = conv_b0
    biasp[:, 1] = conv_b1
    for b in range(4):
        biasp[:, 2 + b] = conv_b2[b * 128 : (b + 1) * 128]
    for ot in range(6):
        biasp[:, 6 + ot] = wh_b[ot * 128 : (ot + 1) * 128]
        biasp[:, 12 + ot] = wt_b[ot * 128 : (ot + 1) * 128]
    shared["biasp"] = biasp
    shared["whp"] = np.ascontiguousarray(
        wh_w.reshape(6, 128, 6, 128).transpose(3, 0, 2, 1).reshape(128, -1)
    ).astype(bf)
    shared["wtp"] = np.ascontiguousarray(
        wt_w.reshape(6, 128, 6, 128).transpose(3, 0, 2, 1).reshape(128, -1)
    ).astype(bf)

    in_maps = []
    for c in range(N_CORES):
        Xc = X[c * WPC : (c + 1) * WPC]                        # [512, 20, 50]
        pair = Xc.reshape(WPC, U, 2, C).transpose(2, 3, 1, 0)  # [2, C, U, 512]
        xpc = np.zeros((128, U * WPC), f)
        xpc[0:C] = pair[0].reshape(C, U * WPC)
        xpc[D1 : D1 + C] = pair[1].reshape(C, U * WPC)
        in_maps.append(dict(xp=xpc.astype(bf), **shared))
    return in_maps


_NC_CACHE = None


def get_nc():
    global _NC_CACHE
    if _NC_CACHE is None:
        _NC_CACHE = build_nc()
    return _NC_CACHE


def kernel(**inputs):
    in_maps = pack_inputs(**{k: np.asarray(v) for k, v in inputs.items()})
    nc = get_nc()
    res = run_bass_kernel_spmd(nc, in_maps, core_ids=list(range(N_CORES)))
    full = np.empty((NW, OUT_DIM), np.float32)
    for c in range(N_CORES):
        full[c * WPC : (c + 1) * WPC] = res.results[c]["out"].T.astype(np.float32)
    return full.reshape(B, S, OUT_DIM)
